# revision 1
# baseline (speedup 1.0000x reference)
"""Multi-head self-attention (causal) Trainium2 kernel, 8-way sharded.

Sharding: core c handles batch b = c//4 and head group g = c%4 (4 of 16
heads). Each core computes q/k/v projections for its head slice, causal
softmax attention, and a partial o_proj ([2048, 1024]); the host sums the
4 partials per batch.

Layouts (per core):
  xT   [1024, 2048]  x[b].T             (d_model on partitions)
  wqT  [1024,  256]  Wq[g*256:(g+1)*256, :].T
  wkT  [1024,  256]
  wvT  [1024,  256]
  woT  [ 256, 1024]  Wo[:, g*256:(g+1)*256].T
  tri  [128, 128]    causal 0/1 triangle (k <= q) for diagonal sub-blocks

Device: all matmuls float32r (~2e-4 rel error, bf16-class PE rate once
the stationary/moving geometry is uniform 128x128). kT head-major stacked
two heads per partition column; qT per-head zero-padded so every S matmul
contracts over K=128 (the other head's half multiplies zeros). V stored
seq-major with an appended ones column so the attention matmul
accumulates the softmax sums in PSUM partition 64; normalization is DVE
reciprocal + GpSimd partition_broadcast + DVE multiply.

Attention runs per (head, query-chunk-pair): key tiles stream outermost,
each kt producing one [128, <=1024] PSUM group (S matmuls), one wide exp
(ACT) straight to f32r SBUF, a 128-col triangle mask on the diagonal
sub-block, and accumulating O matmuls issued one kt-group late so PE
never waits on the exp chain. Diagonal chunks are shrunk to their valid
column range. Query-chunk pairs keep PSUM small enough to triple-buffer
the S groups (6 banks) next to 2 accumulator banks, decoupling ACT from
PE.
"""

import numpy as np

import concourse.bass as bass
import concourse.mybir as mybir
import concourse.tile as tile
from concourse import bacc
from concourse.bass_utils import run_bass_kernel_spmd

P = 128
S = 2048  # sequence length
DM = 1024  # d_model
HD = 64  # head dim
NH_CORE = 4  # heads per core
HSL = NH_CORE * HD  # head slice width = 256
QC = 512  # query chunk
N_QC = S // QC  # 4
N_KT = S // P  # 16 key tiles
KO = DM // P  # 8 k-tiles over d_model

f32 = mybir.dt.float32
f32r = mybir.dt.float32r

_CACHED = {}


def build_program():
    nc = bacc.Bacc("TRN2", target_bir_lowering=False, debug=False)
    xT = nc.declare_dram_parameter("xT", [DM, S], f32, isOutput=False)
    wqT = nc.declare_dram_parameter("wqT", [DM, HSL], f32, isOutput=False)
    wkT = nc.declare_dram_parameter("wkT", [DM, HSL], f32, isOutput=False)
    wvT = nc.declare_dram_parameter("wvT", [DM, HSL], f32, isOutput=False)
    woT = nc.declare_dram_parameter("woT", [HSL, DM], f32, isOutput=False)
    tri = nc.declare_dram_parameter("tri", [P, P], f32r, isOutput=False)
    out = nc.declare_dram_parameter("out", [S, DM], f32, isOutput=True)

    stage_pool = {}

    def load_round(dst_r, dram_ap, stage_shape, dma_eng=None, round_eng="v"):
        """DMA f32 into a staging tile, then copy (rounds) into the f32r
        dst on DVE or ACT."""
        st = stage_pool["p"].tile(stage_shape, f32, tag="stage8k", name="st")
        (dma_eng or nc.sync).dma_start(st[:], dram_ap)
        if round_eng == "v":
            nc.vector.tensor_copy(dst_r, st[:])
        else:
            nc.scalar.activation(
                dst_r, st[:], mybir.ActivationFunctionType.Copy
            )

    with tile.TileContext(nc) as tc:
        with (
            tc.tile_pool(name="persist", bufs=1) as persist,
            tc.tile_pool(name="small", bufs=2) as small,
            tc.tile_pool(name="stg", bufs=4) as stg,
        ):
            stage_pool["p"] = stg
            # ---- persistent tiles
            qTr = persist.tile([P, NH_CORE, S], f32r, tag="qTr")
            kTr = persist.tile([P, 2, S], f32r, tag="kTr")
            vr = persist.tile([P, N_KT, NH_CORE, HD + 1], f32r, tag="vr")
            woTr = persist.tile([P, 2, DM], f32r, tag="woTr")

            # ---- phase 0+1: load x/weights, round, projections
            with (
                tc.tile_pool(name="xw", bufs=1) as xw,
                tc.tile_pool(name="ps_proj", bufs=4, space="PSUM") as ps_proj,
            ):
                # wq first so the q projection can start ASAP; x chunks on
                # separate DMA queues; rounding split across DVE and ACT
                wts = {}
                xTr = xw.tile([P, KO, S], f32r, tag="xTr")

                def load_w(name, dram, round_eng):
                    wr = xw.tile([P, KO, HSL], f32r, tag=f"w{name}r")
                    load_round(
                        wr[:],
                        dram.rearrange("(ko p) m -> p ko m", p=P),
                        [P, KO, HSL],
                        dma_eng=nc.sync,
                        round_eng=round_eng,
                    )
                    wts[name] = wr

                x_qs = [nc.scalar, nc.gpsimd]
                load_w("q", wqT, "v")
                load_round(
                    xTr[:, 0, :], xT[0:P, :], [P, S],
                    dma_eng=x_qs[0], round_eng="v",
                )
                load_w("k", wkT, "v")
                load_w("v", wvT, "v")
                for ko in range(1, KO):
                    load_round(
                        xTr[:, ko, :],
                        xT[ko * P : (ko + 1) * P, :],
                        [P, S],
                        dma_eng=x_qs[ko % 2],
                        round_eng="v",
                    )

                # q/k projections; k head-major stacked [128, 2, 2048],
                # q per-head zero-padded [128, 4, 2048]
                for name in ("q", "k"):
                    wr = wts[name]
                    for mt in range(2):
                        for qc in range(N_QC):
                            ps = ps_proj.tile([P, QC], f32, tag="ps_proj")
                            for ko in range(KO):
                                nc.tensor.matmul(
                                    ps[:],
                                    wr[:, ko, mt * P : (mt + 1) * P],
                                    xTr[:, ko, qc * QC : (qc + 1) * QC],
                                    start=(ko == 0),
                                    stop=(ko == KO - 1),
                                )
                            qsl = slice(qc * QC, (qc + 1) * QC)
                            if name == "k":
                                nc.vector.tensor_copy(
                                    kTr[:, mt, qsl], ps[:]
                                )
                            else:
                                nc.vector.tensor_copy(
                                    qTr[0:HD, 2 * mt, qsl], ps[0:HD, :]
                                )
                                nc.vector.tensor_copy(
                                    qTr[HD:P, 2 * mt + 1, qsl], ps[HD:P, :]
                                )

                # deferred setup (overlaps with projection compute):
                # zero the pad halves of qTr (even heads use partitions 0:64,
                # odd heads 64:128), ones column of V, woT and tri mask loads
                zeros_f = persist.tile([P, 1], f32, tag="zeros")
                nc.vector.memset(zeros_f[:], 0.0)
                nc.vector.tensor_copy(
                    qTr[HD:P, 0::2, :],
                    zeros_f[HD:P, 0:1, None].to_broadcast([HD, 2, S]),
                )
                nc.vector.tensor_copy(
                    qTr[0:HD, 1::2, :],
                    zeros_f[0:HD, 0:1, None].to_broadcast([HD, 2, S]),
                )
                ones_f = persist.tile([P, N_KT * NH_CORE], f32, tag="ones")
                nc.vector.memset(ones_f[:], 1.0)
                nc.vector.tensor_copy(
                    vr[:, :, :, HD].rearrange("p a b -> p (a b)"), ones_f[:]
                )
                load_round(
                    woTr[:],
                    woT.rearrange("(ko p) m -> p ko m", p=P),
                    [P, 2, DM],
                )
                tri_sb = persist.tile([P, P], f32r, tag="tri")
                nc.sync.dma_start(tri_sb[:], tri[:])

                # v projection -> seq-major [128, kt, head, 64(+1)]
                wr = wts["v"]
                for st in range(N_KT):
                    psv = ps_proj.tile([P, QC], f32, tag="ps_proj")
                    ps = psv[:, :HSL]
                    for ko in range(KO):
                        nc.tensor.matmul(
                            ps[:],
                            xTr[:, ko, st * P : (st + 1) * P],
                            wr[:, ko, :],
                            start=(ko == 0),
                            stop=(ko == KO - 1),
                        )
                    nc.vector.tensor_copy(
                        vr[:, st, :, 0:HD],
                        ps[:].rearrange("p (h d) -> p h d", d=HD),
                    )

            with (
                tc.tile_pool(name="phase2", bufs=1) as phase2,
                tc.tile_pool(name="expr", bufs=4) as expr,
                tc.tile_pool(name="outp", bufs=3) as outp,
            ):
                aTr = phase2.tile([P, 2, S], f32r, tag="aTr")

                # ---- phase 2: attention per (head, qc-pair)
                ps_stack = tc.tile_pool(name="ps_s", bufs=2, space="PSUM")
                ps_s_pool = ps_stack.__enter__()
                ps_ot_stack = tc.tile_pool(name="ps_ot", bufs=2, space="PSUM")
                ps_ot_pool = ps_ot_stack.__enter__()

                def normalize(h, qc, ps_ot):
                    hm, hb = h // 2, (h % 2) * HD
                    recip = small.tile([1, QC], f32, tag="recip", name="recip")
                    nc.vector.reciprocal(recip[:], ps_ot[HD : HD + 1, :])
                    bcast = small.tile([HD, QC], f32, tag="bcast", name="bcast")
                    nc.gpsimd.partition_broadcast(bcast[:], recip[:])
                    nc.vector.tensor_mul(
                        aTr[hb : hb + HD, hm, qc * QC : (qc + 1) * QC],
                        ps_ot[0:HD, :],
                        bcast[:],
                    )

                def o_group(h, okt, segs, er_g, ps_ots):
                    # off-diagonal chunks first: the diagonal one also waits
                    # on the DVE triangle mask
                    for qc, c0, o0, w in reversed(segs):
                        nc.tensor.matmul(
                            ps_ots[qc][:, o0:QC],
                            vr[:, okt, h, :],
                            er_g[:, c0 : c0 + w],
                            start=(okt == 0),
                            stop=(okt == 4 * qc + 3),
                        )
                        if okt == 4 * qc + 3:
                            normalize(h, qc, ps_ots[qc])

                for h in range(NH_CORE):
                    hm = h // 2
                    for pr in range(2):
                        qcs = (2 * pr, 2 * pr + 1)
                        ps_ots = {
                            qc: ps_ot_pool.tile(
                                [HD + 1, QC], f32,
                                tag=f"ps_ot{qc % 2}", name="ps_ot",
                            )
                            for qc in qcs
                        }
                        pend = []
                        for kt in range(4 * (qcs[1] + 1)):
                            jd = kt // 4  # diagonal qc for this key tile
                            off = (kt % 4) * P
                            live = [qc for qc in qcs if qc >= jd]
                            ps_g = ps_s_pool.tile(
                                [P, 2 * QC], f32, tag="ps_s", name="ps_g"
                            )
                            er_g = expr.tile(
                                [P, 2 * QC], f32r, tag="er", name="er_g"
                            )
                            # fixed 512-aligned chunk positions: an S
                            # matmul must not cross a PSUM bank boundary
                            segs = []
                            for qc in live:
                                o0 = off if qc == jd else 0
                                c0 = (qc - qcs[0]) * QC + o0
                                segs.append((qc, c0, o0, QC - o0))
                            g0, g1 = segs[0][1], segs[-1][1] + segs[-1][3]
                            for qc, c0, o0, w in segs:
                                nc.tensor.matmul(
                                    ps_g[:, c0 : c0 + w],
                                    kTr[:, hm, kt * P : (kt + 1) * P],
                                    qTr[:, h, qc * QC + o0 : (qc + 1) * QC],
                                    start=True,
                                    stop=True,
                                )
                            nc.scalar.activation(
                                er_g[:, g0:g1],
                                ps_g[:, g0:g1],
                                mybir.ActivationFunctionType.Exp,
                                scale=0.125,
                            )
                            if jd in qcs:
                                c0 = segs[0][1]
                                nc.vector.tensor_mul(
                                    er_g[:, c0 : c0 + P],
                                    er_g[:, c0 : c0 + P],
                                    tri_sb[:],
                                )
                            pend.append((kt, segs, er_g))
                            if len(pend) > 1:
                                okt, osegs, oer = pend.pop(0)
                                o_group(h, okt, osegs, oer, ps_ots)
                        okt, osegs, oer = pend.pop(0)
                        o_group(h, okt, osegs, oer, ps_ots)
                ps_ot_stack.__exit__(None, None, None)
                ps_stack.__exit__(None, None, None)

                # ---- phase 3: partial o_proj [2048, 1024], 1024-wide tiles
                with tc.tile_pool(name="ps_out", bufs=2, space="PSUM") as ps_out:
                    for st in range(N_KT):
                        ps = ps_out.tile([P, DM], f32, tag="ps_out")
                        for nch in range(2):
                            for kt2 in range(2):
                                nc.tensor.matmul(
                                    ps[:, nch * QC : (nch + 1) * QC],
                                    aTr[:, kt2, st * P : (st + 1) * P],
                                    woTr[:, kt2, nch * QC : (nch + 1) * QC],
                                    start=(kt2 == 0),
                                    stop=(kt2 == 1),
                                )
                        ot = outp.tile([P, DM], f32, tag="out_sb")
                        nc.scalar.activation(
                            ot[:], ps[:], mybir.ActivationFunctionType.Copy
                        )
                        nc.sync.dma_start(
                            out[st * P : (st + 1) * P, :], ot[:]
                        )

    nc.compile()
    return nc


def _make_masks():
    k = np.arange(P)[:, None]
    q = np.arange(P)[None, :]
    return (k <= q).astype(np.float32)


def kernel(x, Wq, Wk, Wv, Wo):
    x = np.asarray(x, dtype=np.float32)
    Wq = np.asarray(Wq, dtype=np.float32)
    Wk = np.asarray(Wk, dtype=np.float32)
    Wv = np.asarray(Wv, dtype=np.float32)
    Wo = np.asarray(Wo, dtype=np.float32)
    b, s, dm = x.shape
    assert (b, s, dm) == (2, S, DM), (b, s, dm)

    if "nc" not in _CACHED:
        _CACHED["nc"] = build_program()
    nc = _CACHED["nc"]

    tri = _make_masks()
    in_maps = []
    for c in range(8):
        bi, g = c // 4, c % 4
        sl = slice(g * HSL, (g + 1) * HSL)
        in_maps.append(
            {
                "xT": np.ascontiguousarray(x[bi].T),
                "wqT": np.ascontiguousarray(Wq[sl, :].T),
                "wkT": np.ascontiguousarray(Wk[sl, :].T),
                "wvT": np.ascontiguousarray(Wv[sl, :].T),
                "woT": np.ascontiguousarray(Wo[:, sl].T),
                "tri": tri,
            }
        )

    res = run_bass_kernel_spmd(nc, in_maps, core_ids=list(range(8)))

    out = np.zeros((2, S, DM), dtype=np.float32)
    for c in range(8):
        out[c // 4] += res.results[c]["out"]
    return out



# revision 5
# speedup vs baseline: 1.2924x; 1.2924x over previous
"""Multi-head self-attention (causal) Trainium2 kernel, 8-way sharded.

Sharding: core c handles batch b = c//4 and head group g = c%4 (4 of 16
heads). Each core computes q/k/v projections for its head slice, causal
softmax attention, and a partial o_proj ([2048, 1024]); the host sums the
4 partials per batch.

Single fused instruction stream (no phase barriers): x streams in per
512-query chunk directly as f32r (f32r is bit-compatible with f32, so no
rounding pass), projections for chunks 0-1 run first, then attention on
query-pair 0 with chunk-2/3 projections interleaved as PE filler, then
pair 1 with the pair-0 o_proj interleaved, then the pair-1 o_proj.
Output tiles DMA out as they are produced.

Attention per (head, qc-pair): S matmuls f32r (kT head-major stacked two
heads per partition column, qT per-head zero-padded so every S matmul
contracts over K=128), wide exp (ACT) from PSUM straight to bf16 SBUF,
causal triangle mask as a bf16 4x-mode DVE multiply on diagonal
sub-blocks, O matmuls in bf16 (v seq-major with an appended ones column
so softmax sums accumulate in PSUM partition 64). Normalization is a
single-pass DVE reciprocal_approx_fast + GpSimd partition_broadcast +
DVE multiply into f32r aT; o_proj is f32r.
"""

import numpy as np

import concourse.bass as bass
import concourse.mybir as mybir
import concourse.tile as tile
from concourse import bacc
from concourse.bass_utils import run_bass_kernel_spmd

P = 128
S = 2048  # sequence length
DM = 1024  # d_model
HD = 64  # head dim
NH_CORE = 4  # heads per core
HSL = NH_CORE * HD  # head slice width = 256
QC = 512  # query chunk
N_QC = S // QC  # 4
N_KT = S // P  # 16 key tiles
KO = DM // P  # 8 k-tiles over d_model

f32 = mybir.dt.float32
f32r = mybir.dt.float32r
bf16 = mybir.dt.bfloat16

_CACHED = {}


def build_program():
    nc = bacc.Bacc("TRN2", target_bir_lowering=False, debug=False)
    xT = nc.declare_dram_parameter("xT", [DM, S], f32r, isOutput=False)
    wqT = nc.declare_dram_parameter("wqT", [DM, HSL], f32r, isOutput=False)
    wkT = nc.declare_dram_parameter("wkT", [DM, HSL], f32r, isOutput=False)
    wvT = nc.declare_dram_parameter("wvT", [DM, HSL], f32r, isOutput=False)
    woT = nc.declare_dram_parameter("woT", [HSL, DM], f32r, isOutput=False)
    tri = nc.declare_dram_parameter("tri", [P, P], bf16, isOutput=False)
    out = nc.declare_dram_parameter("out", [S, DM], f32, isOutput=True)

    with tile.TileContext(nc) as tc:
        with (
            tc.tile_pool(name="persist", bufs=1) as persist,
            tc.tile_pool(name="xc", bufs=2) as xcp,
            tc.tile_pool(name="er", bufs=4) as erp,
            tc.tile_pool(name="nrm", bufs=2) as nrm,
            tc.tile_pool(name="outp", bufs=3) as outp,
            tc.tile_pool(name="ps_mm", bufs=2, space="PSUM") as ps_mm,
            tc.tile_pool(name="ps_s", bufs=2, space="PSUM") as ps_sp,
            tc.tile_pool(name="ps_ot", bufs=1, space="PSUM") as ps_otp,
        ):
            # ---- persistent tiles
            qTr = persist.tile([P, NH_CORE, S], f32r, tag="qTr")
            kTr = persist.tile([P, 2, S], f32r, tag="kTr")
            vr = persist.tile([P, N_KT, NH_CORE, HD + 1], bf16, tag="vr")
            woTr = persist.tile([P, 2, DM], f32r, tag="woTr")
            aT = persist.tile([P, 2, S], f32r, tag="aT")
            tri_sb = persist.tile([P, P], bf16, tag="tri")
            wts = {
                n: persist.tile([P, KO, HSL], f32r, tag=f"w{n}", name=f"w{n}")
                for n in ("q", "k", "v")
            }

            # ---- input DMAs: weights first (q projection can start ASAP),
            # then x chunks on alternating queues
            nc.sync.dma_start(
                wts["q"][:], wqT.rearrange("(ko p) m -> p ko m", p=P)
            )
            nc.sync.dma_start(
                wts["k"][:], wkT.rearrange("(ko p) m -> p ko m", p=P)
            )
            nc.sync.dma_start(
                wts["v"][:], wvT.rearrange("(ko p) m -> p ko m", p=P)
            )
            xr = xT.rearrange("(ko p) m -> p ko m", p=P)
            xc = {}

            def load_chunk(c):
                t = xcp.tile([P, KO, QC], f32r, tag="xc", name=f"xc{c}")
                eng = [nc.scalar, nc.gpsimd][c % 2]
                eng.dma_start(t[:], xr[:, :, c * QC : (c + 1) * QC])
                xc[c] = t

            load_chunk(0)
            load_chunk(1)
            nc.sync.dma_start(tri_sb[:], tri[:])
            nc.sync.dma_start(
                woTr[:], woT.rearrange("(kt p) m -> p kt m", p=P)
            )

            # zero the pad halves of qTr (even heads live in partitions
            # 0:64, odd heads in 64:128) and set the ones column of V
            zeros_f = persist.tile([P, 1], f32, tag="zeros")
            nc.vector.memset(zeros_f[:], 0.0)
            nc.gpsimd.tensor_copy(
                qTr[HD:P, 0::2, :],
                zeros_f[HD:P, 0:1, None].to_broadcast([HD, 2, S]),
            )
            nc.gpsimd.tensor_copy(
                qTr[0:HD, 1::2, :],
                zeros_f[0:HD, 0:1, None].to_broadcast([HD, 2, S]),
            )
            ones_f = persist.tile([P, N_KT * NH_CORE], f32, tag="ones")
            nc.vector.memset(ones_f[:], 1.0)
            nc.vector.tensor_copy(
                vr[:, :, :, HD].rearrange("p a b -> p (a b)"), ones_f[:]
            )

            # ---- projection groups (one PSUM accumulation each)
            def proj_qk(name, c, mt):
                ps = ps_mm.tile([P, QC], f32, tag="mm", name="ps_p")
                wr = wts[name]
                for ko in range(KO):
                    nc.tensor.matmul(
                        ps[:],
                        wr[:, ko, mt * P : (mt + 1) * P],
                        xc[c][:, ko, :],
                        start=(ko == 0),
                        stop=(ko == KO - 1),
                    )
                qsl = slice(c * QC, (c + 1) * QC)
                if name == "k":
                    nc.scalar.activation(
                        kTr[:, mt, qsl], ps[:],
                        mybir.ActivationFunctionType.Copy,
                    )
                else:
                    nc.vector.tensor_copy(qTr[0:HD, 2 * mt, qsl], ps[0:HD, :])
                    nc.vector.tensor_copy(
                        qTr[HD:P, 2 * mt + 1, qsl], ps[HD:P, :]
                    )

            def proj_v(c, sti):
                st = 4 * c + sti
                psv = ps_mm.tile([P, QC], f32, tag="mm", name="ps_p")
                ps = psv[:, :HSL]
                for ko in range(KO):
                    nc.tensor.matmul(
                        ps[:],
                        xc[c][:, ko, sti * P : (sti + 1) * P],
                        wts["v"][:, ko, :],
                        start=(ko == 0),
                        stop=(ko == KO - 1),
                    )
                nc.vector.tensor_copy(
                    vr[:, st, :, 0:HD],
                    ps[:].rearrange("p (h d) -> p h d", d=HD),
                )

            def proj_chunk_groups(c):
                return (
                    [lambda mt=mt: proj_qk("q", c, mt) for mt in range(2)]
                    + [lambda mt=mt: proj_qk("k", c, mt) for mt in range(2)]
                    + [lambda s=s: proj_v(c, s) for s in range(4)]
                )

            # ---- attention per (head, qc-pair)
            def normalize(h, qc, ps_ot):
                hm, hb = h // 2, (h % 2) * HD
                sums = nrm.tile([1, QC], f32, tag="sums", name="sums")
                nc.vector.tensor_copy(sums[:], ps_ot[HD : HD + 1, :])
                recip = nrm.tile([1, QC], f32, tag="recip", name="recip")
                nc.vector.reciprocal_approx_fast(recip[:], sums[:])
                bcast = nrm.tile([HD, QC], f32, tag="bcast", name="bcast")
                nc.gpsimd.partition_broadcast(bcast[:], recip[:])
                nc.vector.tensor_mul(
                    aT[hb : hb + HD, hm, qc * QC : (qc + 1) * QC],
                    ps_ot[0:HD, :],
                    bcast[:],
                )

            def o_group(h, okt, segs, er_g, ps_ots):
                # off-diagonal chunks first: the diagonal one also waits
                # on the DVE triangle mask
                for qc, c0, o0, w in reversed(segs):
                    nc.tensor.matmul(
                        ps_ots[qc][:, o0:QC],
                        vr[:, okt, h, :],
                        er_g[:, c0 : c0 + w],
                        start=(okt == 0),
                        stop=(okt == 4 * qc + 3),
                    )
                    if okt == 4 * qc + 3:
                        normalize(h, qc, ps_ots[qc])

            def attn_pair(pr):
                """Generator: one yield per emitted kt-group."""
                qcs = (2 * pr, 2 * pr + 1)
                for h in range(NH_CORE):
                    hm = h // 2
                    ps_ots = {
                        qc: ps_otp.tile(
                            [HD + 1, QC], f32,
                            tag=f"ot{qc % 2}", name="ps_ot",
                        )
                        for qc in qcs
                    }
                    pend = []
                    for kt in range(4 * (qcs[1] + 1)):
                        jd = kt // 4  # diagonal qc for this key tile
                        off = (kt % 4) * P
                        live = [qc for qc in qcs if qc >= jd]
                        ps_g = ps_sp.tile(
                            [P, 2 * QC], f32, tag="s", name="ps_g"
                        )
                        er_g = erp.tile(
                            [P, 2 * QC], bf16, tag="er", name="er_g"
                        )
                        # fixed 512-aligned chunk positions: an S matmul
                        # must not cross a PSUM bank boundary
                        segs = []
                        for qc in live:
                            o0 = off if qc == jd else 0
                            c0 = (qc - qcs[0]) * QC + o0
                            segs.append((qc, c0, o0, QC - o0))
                        g0 = segs[0][1]
                        g1 = segs[-1][1] + segs[-1][3]
                        for qc, c0, o0, w in segs:
                            nc.tensor.matmul(
                                ps_g[:, c0 : c0 + w],
                                kTr[:, hm, kt * P : (kt + 1) * P],
                                qTr[:, h, qc * QC + o0 : (qc + 1) * QC],
                                start=True,
                                stop=True,
                            )
                        nc.scalar.activation(
                            er_g[:, g0:g1],
                            ps_g[:, g0:g1],
                            mybir.ActivationFunctionType.Exp,
                            scale=0.125,
                        )
                        if jd in qcs:
                            c0 = segs[0][1]
                            nc.vector.tensor_mul(
                                er_g[:, c0 : c0 + P],
                                er_g[:, c0 : c0 + P],
                                tri_sb[:],
                            )
                        pend.append((kt, segs, er_g))
                        if len(pend) > 1:
                            okt, osegs, oer = pend.pop(0)
                            o_group(h, okt, osegs, oer, ps_ots)
                        yield
                    okt, osegs, oer = pend.pop(0)
                    o_group(h, okt, osegs, oer, ps_ots)
                    yield

            # ---- partial o_proj, one 128-seq tile at a time, DMA'd out
            def oproj_st(st):
                stg = outp.tile([P, DM], f32, tag="out_sb", name="ot_sb")
                for nch in range(2):
                    ps = ps_mm.tile([P, QC], f32, tag="mm", name="ps_o")
                    for kt2 in range(2):
                        nc.tensor.matmul(
                            ps[:],
                            aT[:, kt2, st * P : (st + 1) * P],
                            woTr[:, kt2, nch * QC : (nch + 1) * QC],
                            start=(kt2 == 0),
                            stop=(kt2 == 1),
                        )
                    if nch == 0:
                        nc.vector.tensor_copy(stg[:, nch * QC :][:, :QC], ps[:])
                    else:
                        nc.scalar.activation(
                            stg[:, nch * QC :][:, :QC], ps[:],
                            mybir.ActivationFunctionType.Copy,
                        )
                [nc.sync, nc.gpsimd][st % 2].dma_start(
                    out[st * P : (st + 1) * P, :], stg[:]
                )

            # ---- fused schedule
            for grp in proj_chunk_groups(0):
                grp()
            load_chunk(2)
            for grp in proj_chunk_groups(1):
                grp()
            load_chunk(3)

            # pair 0 with chunk-2/3 projections as PE filler
            filler = proj_chunk_groups(2) + proj_chunk_groups(3)
            for step in attn_pair(0):
                if filler:
                    filler.pop(0)()
            while filler:
                filler.pop(0)()

            # pair 1 with the pair-0 o_proj as PE filler
            filler = [lambda st=st: oproj_st(st) for st in range(8)]
            for i, step in enumerate(attn_pair(1)):
                if filler and i % 7 == 6:
                    filler.pop(0)()
            while filler:
                filler.pop(0)()

            for st in range(8, 16):
                oproj_st(st)

    nc.compile()
    return nc


def _make_masks():
    k = np.arange(P)[:, None]
    q = np.arange(P)[None, :]
    return (k <= q).astype(np.float32)


def make_in_maps(x, Wq, Wk, Wv, Wo):
    import ml_dtypes

    tri = _make_masks().astype(ml_dtypes.bfloat16)
    in_maps = []
    for c in range(8):
        bi, g = c // 4, c % 4
        sl = slice(g * HSL, (g + 1) * HSL)
        in_maps.append(
            {
                "xT": np.ascontiguousarray(x[bi].T),
                "wqT": np.ascontiguousarray(Wq[sl, :].T),
                "wkT": np.ascontiguousarray(Wk[sl, :].T),
                "wvT": np.ascontiguousarray(Wv[sl, :].T),
                "woT": np.ascontiguousarray(Wo[:, sl].T),
                "tri": tri,
            }
        )
    return in_maps


def kernel(x, Wq, Wk, Wv, Wo):
    x = np.asarray(x, dtype=np.float32)
    Wq = np.asarray(Wq, dtype=np.float32)
    Wk = np.asarray(Wk, dtype=np.float32)
    Wv = np.asarray(Wv, dtype=np.float32)
    Wo = np.asarray(Wo, dtype=np.float32)
    b, s, dm = x.shape
    assert (b, s, dm) == (2, S, DM), (b, s, dm)

    if "nc" not in _CACHED:
        _CACHED["nc"] = build_program()
    nc = _CACHED["nc"]

    in_maps = make_in_maps(x, Wq, Wk, Wv, Wo)
    res = run_bass_kernel_spmd(nc, in_maps, core_ids=list(range(8)))

    out = np.zeros((2, S, DM), dtype=np.float32)
    for c in range(8):
        out[c // 4] += res.results[c]["out"]
    return out
